# revision 10
# baseline (speedup 1.0000x reference)
"""Trainium2 Bass kernel for LoRA-segmented linear layer.

Computes y = x @ W^T + bias + scalings[e] * (x_e @ A_e^T) @ B_e^T
where x is split into 8 equal contiguous token segments (one per adapter).

Sharding: data-parallel over tokens; core e gets segment e (4096 tokens),
which exactly matches adapter e, so all LoRA work is core-local.

Per-core device kernel:
  0. Warmup matmuls on a zeroed tile keep the PE busy from t=0 so the HAM
     clock gate reaches 8/8 (2.4 GHz) before real work arrives, and stays
     there through the DMA-bound startup phase (any PE idle gap > ~2us
     re-throttles the clock to 1.2 GHz and semaphore-paced work never
     re-warms it, so the whole startup must be gap-free).
  1. Fold LoRA into an effective weight on-device:
       W_eff^T = W^T + A_e^T @ (s_e * B_e^T)
     The rank-16 product goes to PSUM on the PE; evacuation to SBUF is
     split across two engines to keep it off the critical path:
       - d_out columns 0-1023: DVE tensor_add(psum, W_tile)
       - d_out columns 1024-2047: an identity matmul accumulates the W tile
         into PSUM on the (otherwise idle) PE, then the scalar/ACT engine
         copies psum -> sbuf (bf16 downcast).
  2. Dense GEMM y_e = x_e @ W_eff^T + bias, tiled as:
       stationary = x^T tile [128(d) x 128(tok)], moving = W_eff^T [128 x 512]
       PSUM accumulates fp32 over the 16 k-tiles; DVE adds bias; one
       1 MB DMA per 128-token row block writes y out on the scalar ring.

The host pre-reshapes x and W into the exact SBUF tile layouts
([partition, k-tile-major free dim]) so every device DMA is a plain 2D
contiguous slice — 4-16KB per partition per transfer, fast HWDGE
descriptors, no 3D access patterns (those cost >10us of trigger time and
poison the 8 global DMA lane semaphores with false ordering deps).
Ring priority: inputs (at/sbt/ident -> W waves -> x chunks) in strict
FIFO order on the sync ring; y output on the scalar ring.

After TileContext exit we deduplicate InstLdweights: the tile legalizer
emits one LDWEIGHTS per matmul, but consecutive matmuls that share the
same stationary AP (the 4 d_out chunks per x-tile) only need the first.
This cuts the PE issue gap from ~259ns/MM to the ~216ns streaming floor.
"""

import numpy as np
import ml_dtypes

# Problem geometry (hardcoded per contest contract).
N_TOK, D_IN, D_OUT, E, R = 32768, 2048, 2048, 8, 16
S = N_TOK // E          # tokens per core / segment: 4096
P = 128                 # partitions
NK = D_IN // P          # 16 contraction tiles
TCH = 256               # token chunk (per x DMA)
NCH = S // TCH          # 16 token chunks per core
M_PER = TCH // P        # 2 m-subtiles (of 128 tokens) per chunk
OC = 512                # dout chunk (matmul moving free dim; one PSUM bank)
NOC = D_OUT // OC       # 4 dout chunks
WAVE = 2                # k-tiles per W DMA wave
NWAVE = NK // WAVE      # 8 waves
N_WARM = 14             # warmup matmuls bridge t=0 to the first W arrival

_PROGRAM = None         # cached Bass program
LAST_RESULTS = None     # BassKernelResults of the most recent run (for profiling)


def _dedup_ldweights(nc):
    """Remove InstLdweights that reload the stationary AP already resident
    (identical AP string, only matmults in between). Any waits on a removed
    LDW are moved onto the next matmult. Must run after TileContext exit and
    before nc.finalize()."""
    import concourse.mybir as mybir

    n_removed = 0
    for fn in nc.m.functions:
        for bb in fn.blocks:
            insts = list(bb.instructions)
            keep = []
            last_key = None
            pending_waits = []
            removed_here = False
            for i in insts:
                tn = type(i).__name__
                if tn == "InstLdweights":
                    key = (str(i.ins[0]), str(i.tile_position))
                    if key == last_key:
                        si = i.sync_info
                        if si is not None and si.on_wait:
                            pending_waits.extend(si.on_wait)
                        n_removed += 1
                        removed_here = True
                        continue
                    last_key = key
                elif tn == "InstMatmult":
                    if pending_waits:
                        si = i.sync_info
                        if si is None:
                            i.sync_info = mybir.SyncInfo(
                                on_wait=list(pending_waits), on_update=[])
                        else:
                            si.on_wait = list(si.on_wait) + pending_waits
                            i.sync_info = si
                        pending_waits = []
                else:
                    last_key = None
                keep.append(i)
            assert not pending_waits
            if removed_here:
                bb.instructions = keep
    return n_removed


def _build_program(in_dt_name="bfloat16"):
    from contextlib import ExitStack

    import concourse.mybir as mybir
    import concourse.tile as tile
    from concourse import bacc

    in_dt = getattr(mybir.dt, in_dt_name)
    f32 = mybir.dt.float32
    COPY = mybir.ActivationFunctionType.Copy

    nc = bacc.Bacc(trn_type="TRN2")

    # Host-prepped layouts: free dim is k-tile-major so device DMAs are
    # plain contiguous 2D slices.
    #   xk[p, t*NK*TCH + k*TCH + c] = x[t*TCH + c, k*P + p]
    #   wk[p, k*D_OUT + c]          = W^T[k*P + p, c]
    xk_d = nc.dram_tensor("xk", [P, S * NK], in_dt, kind="ExternalInput")
    wk_d = nc.dram_tensor("wk", [P, NK * D_OUT], in_dt, kind="ExternalInput")
    bias_d = nc.dram_tensor("bias", [D_OUT], f32, kind="ExternalInput")
    at = nc.dram_tensor("at", [R, D_IN], in_dt, kind="ExternalInput")
    sbt = nc.dram_tensor("sbt", [R, D_OUT], in_dt, kind="ExternalInput")
    ident_d = nc.dram_tensor("ident", [P, P], in_dt, kind="ExternalInput")
    y = nc.dram_tensor("y", [S, D_OUT], f32, kind="ExternalOutput")

    with ExitStack() as ctx:
        tc = ctx.enter_context(tile.TileContext(nc))
        persist = ctx.enter_context(tc.tile_pool(name="persist", bufs=1))
        wpool = ctx.enter_context(tc.tile_pool(name="wpool", bufs=7))
        xp = ctx.enter_context(tc.tile_pool(name="xp", bufs=4))
        outp = ctx.enter_context(tc.tile_pool(name="outp", bufs=2))
        psum = ctx.enter_context(tc.tile_pool(name="psum", bufs=8, space="PSUM"))

        # --- HAM warmup: PE busy from t=0 so the clock ungates early ---
        warm = persist.tile([P, OC], in_dt, tag="warm", name="warm_sb")
        nc.vector.memset(warm, 0.0)
        wps = psum.tile([P, OC], f32, tag="ps", name="warm_ps")
        for i in range(N_WARM):
            nc.tensor.matmul(wps, warm[:, :P], warm[:], start=True, stop=True)

        # --- persistent small tensors (first on the sync ring) ---
        at_sb = persist.tile([R, D_IN], in_dt, tag="at", name="at_sb")
        nc.sync.dma_start(out=at_sb, in_=at[:])
        sbt_sb = persist.tile([R, D_OUT], in_dt, tag="sbt", name="sbt_sb")
        nc.sync.dma_start(out=sbt_sb, in_=sbt[:])
        ident_sb = persist.tile([P, P], in_dt, tag="ident", name="ident_sb")
        nc.sync.dma_start(out=ident_sb, in_=ident_d[:])
        bias_sb = persist.tile([P, D_OUT], f32, tag="bias", name="bias_sb")
        # stride-0 partition broadcast must go via SW DGE (gpsimd), not HW DGE
        nc.gpsimd.dma_start(out=bias_sb, in_=bias_d[:].partition_broadcast(P))

        # --- fold LoRA into effective weight: weff[k] = wt[k] + A^T_k @ sBt ---
        weff = []
        for wave in range(NWAVE):
            wv = wpool.tile([P, WAVE * D_OUT], in_dt, tag="wv", name=f"wv_{wave}")
            nc.sync.dma_start(
                out=wv,
                in_=wk_d[:, wave * WAVE * D_OUT:(wave + 1) * WAVE * D_OUT],
            )
            for j in range(WAVE):
                k = wave * WAVE + j
                we = persist.tile([P, D_OUT], in_dt, tag=f"weff{k}",
                                  name=f"weff_{k}")
                pss = [
                    psum.tile([P, OC], f32, tag="ps", name=f"pps_{k}_{oc}")
                    for oc in range(NOC)
                ]
                atk = at_sb[:, k * P:(k + 1) * P]
                for oc in range(NOC):
                    nc.tensor.matmul(
                        pss[oc],
                        atk,
                        sbt_sb[:, oc * OC:(oc + 1) * OC],
                        start=True,
                        stop=(oc < 2),
                    )
                for oc in (2, 3):
                    nc.tensor.matmul(
                        pss[oc],
                        ident_sb[:],
                        wv[:, j * D_OUT + oc * OC:j * D_OUT + (oc + 1) * OC],
                        start=False,
                        stop=True,
                    )
                for oc in (0, 1):
                    nc.vector.tensor_add(
                        we[:, oc * OC:(oc + 1) * OC],
                        pss[oc],
                        wv[:, j * D_OUT + oc * OC:j * D_OUT + (oc + 1) * OC],
                    )
                for oc in (2, 3):
                    nc.scalar.activation(
                        we[:, oc * OC:(oc + 1) * OC], pss[oc], COPY,
                    )
                weff.append(we)

        # --- main GEMM over token chunks ---
        XSZ = NK * TCH  # flat x elems per chunk (per partition)
        for t in range(NCH):
            xch = xp.tile([P, XSZ], in_dt, tag="xch", name=f"xch_{t}")
            nc.sync.dma_start(out=xch, in_=xk_d[:, t * XSZ:(t + 1) * XSZ])
            for m in range(M_PER):
                pss = [
                    psum.tile([P, OC], f32, tag="ps", name=f"ps_{t}_{m}_{oc}")
                    for oc in range(NOC)
                ]
                for k in range(NK):
                    lhsT = xch[:, k * TCH + m * P:k * TCH + (m + 1) * P]
                    for oc in range(NOC):
                        nc.tensor.matmul(
                            pss[oc],
                            lhsT,
                            weff[k][:, oc * OC:(oc + 1) * OC],
                            start=(k == 0),
                            stop=(k == NK - 1),
                        )
                row0 = (t * M_PER + m) * P
                ob = outp.tile([P, D_OUT], f32, tag="ob", name=f"ob_{t}_{m}")
                last = (t == NCH - 1 and m == M_PER - 1)
                for oc in range(NOC):
                    nc.vector.tensor_add(
                        ob[:, oc * OC:(oc + 1) * OC], pss[oc],
                        bias_sb[:, oc * OC:(oc + 1) * OC]
                    )
                    if last:
                        nc.scalar.dma_start(
                            out=y[row0:row0 + P, oc * OC:(oc + 1) * OC],
                            in_=ob[:, oc * OC:(oc + 1) * OC],
                        )
                if not last:
                    nc.scalar.dma_start(out=y[row0:row0 + P, :], in_=ob)

    _dedup_ldweights(nc)
    return nc


def _get_program():
    global _PROGRAM
    if _PROGRAM is None:
        _PROGRAM = _build_program()
        # run_bass_via_pjrt does not finalize; Bacc's compile passes
        # (register alloc, wait legalization) run here.
        _PROGRAM.finalize()
    return _PROGRAM


def kernel(x, W, bias, lora_a, lora_b, scalings, trace=False):
    global LAST_RESULTS
    from concourse.bass_utils import run_bass_kernel_spmd

    assert x.shape == (N_TOK, D_IN) and W.shape == (D_OUT, D_IN)
    bf16 = ml_dtypes.bfloat16

    # Host-side layout prep (not on the device critical path).
    xb = x.astype(bf16)                                            # [N, D_IN]
    # xk[e][p, (t,k,c)] = x[e*S + t*TCH + c, k*P + p]
    xk_all = np.ascontiguousarray(
        xb.reshape(E, NCH, TCH, NK, P).transpose(0, 4, 1, 3, 2)
        .reshape(E, P, S * NK)
    )
    # wk[p, (k,c)] = W^T[k*P + p, c] = W[c, k*P + p]
    wk = np.ascontiguousarray(
        W.astype(bf16).T.reshape(NK, P, D_OUT).transpose(1, 0, 2)
        .reshape(P, NK * D_OUT)
    )
    at_all = lora_a.astype(bf16)                                   # [E, R, D_IN]
    sbt_all = np.ascontiguousarray(
        (lora_b.astype(np.float64) * scalings[:, None, None].astype(np.float64))
        .transpose(0, 2, 1)
    ).astype(bf16)                                                 # [E, R, D_OUT]
    bias32 = np.ascontiguousarray(bias.astype(np.float32))
    ident = np.eye(P, dtype=bf16)

    in_maps = []
    for e in range(E):
        in_maps.append(
            {
                "xk": xk_all[e],
                "wk": wk,
                "bias": bias32,
                "at": np.ascontiguousarray(at_all[e]),
                "sbt": np.ascontiguousarray(sbt_all[e]),
                "ident": ident,
            }
        )

    nc = _get_program()
    res = run_bass_kernel_spmd(nc, in_maps, core_ids=list(range(E)), trace=trace)
    LAST_RESULTS = res
    out = np.concatenate([r["y"] for r in res.results], axis=0)
    return out.astype(np.float32)
